# revision 11
# baseline (speedup 1.0000x reference)
"""LinearWithLoRA on 8 TRN2 NeuronCores — split-K fp8-DoubleRow/fp16 hybrid.

y = x @ W.T + b + 2.0 * (x @ A.T) @ B.T
  x: [4, 2048, 2048] f32, W: [2048, 2048], b: [2048], A: [16, 2048], B: [2048, 16]

Strategy:
- LoRA merge on host: W' = W + 2.0 * B @ A, so the device computes
  x @ W'.T + b. Data-parallel over tokens (8192 -> 1024/core), W'
  replicated, no collectives.
- The PE's fp8 DoubleRow mode (both operands e4m3) retires 2 k-planes per
  512-cycle instruction = 2x the bf16/fp16/fp32r rate. Pure fp8 is too
  noisy for the 2e-2 gate (2.41e-2), so split the contraction: 10 of the
  16 k-planes run as 5 DoubleRow fp8 instructions, the remaining 6 planes
  run exact in fp16. Same-scale quantization (x*32, W'*1024, powers of
  two so dequant is exact) lets both parts accumulate into one PSUM bank.
  Measured rel err 1.906e-2; PE cost = (5 + 6)/16 = 0.69 of a full-K
  one-dtype pass (~76us/core vs 109us floor for any single-dtype scheme).
- Operand swap: W chunks are stationary, x moving, so PSUM tiles are
  [out_ch(partition), tokens(free)] and the bias varies along partitions:
  eviction is ONE scalar-engine op per tile,
  out = Identity(psum * 2^-15 + bias[p]). Output is written transposed
  and re-transposed on host.
- All of W' (fp8+fp16 halves, ~5.5 MiB) and the x shard stay resident in
  SBUF. Matmuls are issued k-plane-outer over groups of 8 PSUM tiles so
  the PE consumes operands in exact DMA arrival order: w8 chunks for the
  first group, then x8 plane-by-plane, then w16/x16 — the PE starts after
  ~0.75 MiB instead of waiting for the full 8.25 MiB input stream.
"""

import numpy as np
import ml_dtypes

import concourse.bass as bass
import concourse.mybir as mybir
import concourse.tile as tile
from concourse import bacc
from concourse.bass import ds, ts
from concourse.bass_utils import run_bass_kernel_spmd

B, S, D_IN, D_OUT, R = 4, 2048, 2048, 2048, 16
SCALING = 32.0 / 16.0
N_CORES = 8
TOK = B * S  # 8192
TOK_SHARD = TOK // N_CORES  # 1024
P = 128
KF8 = 10  # k-planes (128 each) done in fp8 DoubleRow (must be even)
KF16 = 16 - KF8  # k-planes done in fp16
J8 = KF8 // 2  # DoubleRow instructions per tile
T_CHUNK = 512  # psum bank: 512 f32 per partition
T_CHUNKS = TOK_SHARD // T_CHUNK  # 2
O_TILES = D_OUT // P  # 16 out-channel tiles
OT_GROUP = 4  # out-tiles per psum group (x T_CHUNKS = 8 banks)

SX = 32.0  # x scale (2^5)
SW = 1024.0  # W scale (2^10)
EVICT_SCALE = 1.0 / (SX * SW)  # 2^-15, exact in fp32

E4M3 = ml_dtypes.float8_e4m3

_nc_cache = {}


def _build():
    f32 = mybir.dt.float32
    f8 = mybir.dt.float8e4
    f16 = mybir.dt.float16
    DR = mybir.MatmulPerfMode.DoubleRow
    IDENT = mybir.ActivationFunctionType.Identity

    nc = bacc.Bacc(None, target_bir_lowering=False)
    x8d = nc.dram_tensor("x8", [P, J8, 2, TOK_SHARD], f8, kind="ExternalInput")
    x16d = nc.dram_tensor("x16", [P, KF16, TOK_SHARD], f16, kind="ExternalInput")
    w8d = nc.dram_tensor("w8", [O_TILES, P, J8, 2, P], f8, kind="ExternalInput")
    w16d = nc.dram_tensor("w16", [O_TILES, P, KF16, P], f16, kind="ExternalInput")
    biasd = nc.dram_tensor("bias", [P, O_TILES], f32, kind="ExternalInput")
    out = nc.dram_tensor("outT", [D_OUT, TOK_SHARD], f16, kind="ExternalOutput")

    n_groups = O_TILES // OT_GROUP

    with tile.TileContext(nc) as tc:
        with (
            tc.tile_pool(name="xpool", bufs=1) as xpool,
            tc.tile_pool(name="wpool", bufs=1) as wpool,
            tc.tile_pool(name="cpool", bufs=1) as cpool,
            tc.tile_pool(name="opool", bufs=8) as opool,
            tc.tile_pool(name="ppool", bufs=8, space="PSUM") as ppool,
        ):
            x8t = xpool.tile([P, J8, 2, TOK_SHARD], f8)
            x16t = xpool.tile([P, KF16, TOK_SHARD], f16)
            w8t = wpool.tile([P, O_TILES, J8, 2, P], f8)
            w16t = wpool.tile([P, O_TILES, KF16, P], f16)
            bias_t = cpool.tile([P, O_TILES], f32)

            # Partition-major views of the W inputs so a whole 4-chunk group
            # loads in ONE descriptor issue (the Sync engine issues DMA
            # descriptors serially at ~650ns each — issue count matters).
            w8r = w8d.rearrange("o p j t i -> p o j t i")
            w16r = w16d.rearrange("o p k i -> p o k i")

            def load_w_group(g):
                sl = ds(g * OT_GROUP, OT_GROUP)
                nc.sync.dma_start(out=w8t[:, sl, :, :, :], in_=w8r[:, sl, :, :, :])
                nc.sync.dma_start(out=w16t[:, sl, :, :], in_=w16r[:, sl, :, :])

            # DMA issue order == PE consumption order (HWDGE is FIFO):
            # group-0 fp8 weights, x8 plane-by-plane, group-0 fp16 weights,
            # x16 plane-by-plane, bias, then the remaining W groups.
            nc.sync.dma_start(
                out=w8t[:, ds(0, 1), ds(0, 1), :, :],
                in_=w8r[:, ds(0, 1), ds(0, 1), :, :],
            )
            nc.sync.dma_start(
                out=x8t[:, 0, :, ts(0, T_CHUNK)], in_=x8d[:, 0, :, ts(0, T_CHUNK)]
            )
            nc.sync.dma_start(
                out=x8t[:, 0, :, ts(1, T_CHUNK)], in_=x8d[:, 0, :, ts(1, T_CHUNK)]
            )
            nc.sync.dma_start(
                out=w8t[:, ds(0, 1), ds(1, J8 - 1), :, :],
                in_=w8r[:, ds(0, 1), ds(1, J8 - 1), :, :],
            )
            nc.sync.dma_start(
                out=w8t[:, ds(1, OT_GROUP - 1), :, :, :],
                in_=w8r[:, ds(1, OT_GROUP - 1), :, :, :],
            )
            for j in range(1, J8):
                nc.sync.dma_start(out=x8t[:, j, :, :], in_=x8d[:, j, :, :])
            nc.sync.dma_start(
                out=w16t[:, ds(0, OT_GROUP), :, :], in_=w16r[:, ds(0, OT_GROUP), :, :]
            )
            for k in range(KF16):
                nc.sync.dma_start(out=x16t[:, k, :], in_=x16d[:, k, :])
            nc.sync.dma_start(out=bias_t[:], in_=biasd[:, :])
            for g in range(1, n_groups):
                load_w_group(g)

            def mm8(ps, ot, j, tt, start):
                nc.tensor.matmul(
                    ps[:],
                    w8t[:, ot, j, :, :],
                    x8t[:, j, :, ts(tt, T_CHUNK)],
                    start=start,
                    stop=False,
                    perf_mode=DR,
                )

            def mm16(ps, ot, k, tt):
                nc.tensor.matmul(
                    ps[:],
                    w16t[:, ot, k, :],
                    x16t[:, k, ts(tt, T_CHUNK)],
                    start=False,
                    stop=(k == KF16 - 1),
                )

            def evict(ps, st, ot, tt):
                nc.scalar.activation(
                    st[:, ts(tt, T_CHUNK)],
                    ps[:],
                    IDENT,
                    bias=bias_t[:, ds(ot, 1)],
                    scale=EVICT_SCALE,
                )

            for g in range(n_groups):
                tiles = [
                    (ot, tt)
                    for ot in range(g * OT_GROUP, (g + 1) * OT_GROUP)
                    for tt in range(T_CHUNKS)
                ]
                if g == 0:
                    # k-plane-outer: all 8 tiles advance together through
                    # the operand stream, consuming each freshly arrived x
                    # plane with 8 back-to-back matmuls (DMA-paced phase).
                    pss = [
                        ppool.tile([P, T_CHUNK], f32, tag="ps", name=f"ps{i}")
                        for i in range(len(tiles))
                    ]
                    for j in range(J8):
                        for i, (ot, tt) in enumerate(tiles):
                            mm8(pss[i], ot, j, tt, start=(j == 0))
                    for k in range(KF16):
                        for i, (ot, tt) in enumerate(tiles):
                            mm16(pss[i], ot, k, tt)
                    stage = {}
                    for i, (ot, tt) in enumerate(tiles):
                        if ot not in stage:
                            stage[ot] = opool.tile([P, TOK_SHARD], f16, name="ot")
                        evict(pss[i], stage[ot], ot, tt)
                    for ot, st in stage.items():
                        nc.sync.dma_start(out=out[ts(ot, P), :], in_=st[:])
                else:
                    # tile-major: each psum tile completes as early as
                    # possible so evictions and output stores stagger into
                    # the matmul stream instead of bunching at the end.
                    for ot in range(g * OT_GROUP, (g + 1) * OT_GROUP):
                        st = opool.tile([P, TOK_SHARD], f16, name="ot")
                        last = ot == O_TILES - 1
                        for tt in range(T_CHUNKS):
                            ps = ppool.tile([P, T_CHUNK], f32, tag="ps", name="ps")
                            for j in range(J8):
                                mm8(ps, ot, j, tt, start=(j == 0))
                            for k in range(KF16):
                                mm16(ps, ot, k, tt)
                            evict(ps, st, ot, tt)
                            if last:
                                # final tile: store each half as soon as it
                                # evicts so the NEFF-end queue drain waits on
                                # a 256KB transfer, not 512KB
                                nc.sync.dma_start(
                                    out=out[ts(ot, P), ts(tt, T_CHUNK)],
                                    in_=st[:, ts(tt, T_CHUNK)],
                                )
                        if not last:
                            nc.sync.dma_start(out=out[ts(ot, P), :], in_=st[:])

    nc.compile()
    return nc


def _make_in_maps(x, W, b, lora_A, lora_B):
    # LoRA merge: W' = W + scaling * B @ A  (exact fp32 host math)
    w_merged = W + SCALING * (lora_B @ lora_A)

    KC = KF8 * P  # k cut point
    ws = w_merged.T * SW  # [D_IN, D_OUT]
    w8 = np.ascontiguousarray(
        ws[:KC].astype(E4M3).reshape(J8, 2, P, O_TILES, P).transpose(3, 2, 0, 1, 4)
    )
    w16 = np.ascontiguousarray(
        ws[KC:].astype(np.float16).reshape(KF16, P, O_TILES, P).transpose(2, 1, 0, 3)
    )

    xs = x.reshape(TOK, D_IN).T * SX  # [D_IN, TOK]
    xq8 = xs[:KC].astype(E4M3)
    xq16 = xs[KC:].astype(np.float16)

    bias = np.ascontiguousarray(b.reshape(O_TILES, P).T)  # [P, O_TILES]

    def shard8(i):
        s = xq8[:, i * TOK_SHARD : (i + 1) * TOK_SHARD]
        return np.ascontiguousarray(
            s.reshape(J8, 2, P, TOK_SHARD).transpose(2, 0, 1, 3)
        )

    def shard16(i):
        s = xq16[:, i * TOK_SHARD : (i + 1) * TOK_SHARD]
        return np.ascontiguousarray(
            s.reshape(KF16, P, TOK_SHARD).transpose(1, 0, 2)
        )

    return [
        {
            "x8": shard8(i),
            "x16": shard16(i),
            "w8": w8,
            "w16": w16,
            "bias": bias,
        }
        for i in range(N_CORES)
    ]


def kernel(x, W, b, lora_A, lora_B):
    x = np.asarray(x, dtype=np.float32)
    W = np.asarray(W, dtype=np.float32)
    b = np.asarray(b, dtype=np.float32)
    lora_A = np.asarray(lora_A, dtype=np.float32)
    lora_B = np.asarray(lora_B, dtype=np.float32)

    if "main" not in _nc_cache:
        _nc_cache["main"] = _build()
    nc = _nc_cache["main"]

    in_maps = _make_in_maps(x, W, b, lora_A, lora_B)
    res = run_bass_kernel_spmd(nc, in_maps, list(range(N_CORES)))
    out = np.concatenate(
        [res.results[i]["outT"].astype(np.float32).T for i in range(N_CORES)],
        axis=0,
    )
    return np.ascontiguousarray(out).reshape(B, S, D_OUT)


# revision 13
# speedup vs baseline: 1.0130x; 1.0130x over previous
"""LinearWithLoRA on 8 TRN2 NeuronCores — split-K fp8-DoubleRow/fp16 hybrid.

y = x @ W.T + b + 2.0 * (x @ A.T) @ B.T
  x: [4, 2048, 2048] f32, W: [2048, 2048], b: [2048], A: [16, 2048], B: [2048, 16]

Strategy:
- LoRA merge on host: W' = W + 2.0 * B @ A, so the device computes
  x @ W'.T + b. Data-parallel over tokens (8192 -> 1024/core), W'
  replicated, no collectives.
- The PE's fp8 DoubleRow mode (both operands e4m3) retires 2 k-planes per
  512-cycle instruction = 2x the bf16/fp16/fp32r rate. Pure fp8 is too
  noisy for the 2e-2 gate (2.41e-2), so split the contraction: 10 of the
  16 k-planes run as 5 DoubleRow fp8 instructions, the remaining 6 planes
  run exact in fp16. Same-scale quantization (x*32, W'*1024, powers of
  two so dequant is exact) lets both parts accumulate into one PSUM bank.
  Measured rel err 1.906e-2; PE cost = (5 + 6)/16 = 0.69 of a full-K
  one-dtype pass (~76us/core vs 109us floor for any single-dtype scheme).
- Operand swap: W chunks are stationary, x moving, so PSUM tiles are
  [out_ch(partition), tokens(free)] and the bias varies along partitions:
  eviction is ONE scalar-engine op per tile,
  out = Identity(psum * 2^-15 + bias[p]), with the vector engine left
  idle. Output is stored as f16 (halves output DMA; adds ~1e-5 rel err),
  transposed, and upcast/re-transposed on host.
- All of W' (fp8+fp16 halves, ~5.5 MiB) and the x shard stay resident in
  SBUF. Matmuls are issued k-plane-outer over groups of 8 PSUM tiles so
  the PE consumes operands in exact DMA arrival order: w8 chunks for the
  first group, then x8 plane-by-plane, then w16/x16 — the PE starts after
  ~0.75 MiB instead of waiting for the full 8.25 MiB input stream.
"""

import numpy as np
import ml_dtypes

import concourse.bass as bass
import concourse.mybir as mybir
import concourse.tile as tile
from concourse import bacc
from concourse.bass import ds, ts
from concourse.bass_utils import run_bass_kernel_spmd

B, S, D_IN, D_OUT, R = 4, 2048, 2048, 2048, 16
SCALING = 32.0 / 16.0
N_CORES = 8
TOK = B * S  # 8192
TOK_SHARD = TOK // N_CORES  # 1024
P = 128
KF8 = 10  # k-planes (128 each) done in fp8 DoubleRow (must be even)
KF16 = 16 - KF8  # k-planes done in fp16
J8 = KF8 // 2  # DoubleRow instructions per tile
T_CHUNK = 512  # psum bank: 512 f32 per partition
T_CHUNKS = TOK_SHARD // T_CHUNK  # 2
O_TILES = D_OUT // P  # 16 out-channel tiles
OT_GROUP = 4  # out-tiles per psum group (x T_CHUNKS = 8 banks)

SX = 32.0  # x scale (2^5)
SW = 1024.0  # W scale (2^10)
EVICT_SCALE = 1.0 / (SX * SW)  # 2^-15, exact in fp32

E4M3 = ml_dtypes.float8_e4m3

_nc_cache = {}


def _build():
    f32 = mybir.dt.float32
    f8 = mybir.dt.float8e4
    f16 = mybir.dt.float16
    DR = mybir.MatmulPerfMode.DoubleRow
    IDENT = mybir.ActivationFunctionType.Identity

    nc = bacc.Bacc(None, target_bir_lowering=False)
    x8d = nc.dram_tensor("x8", [P, J8, 2, TOK_SHARD], f8, kind="ExternalInput")
    x16d = nc.dram_tensor("x16", [P, KF16, TOK_SHARD], f16, kind="ExternalInput")
    w8d = nc.dram_tensor("w8", [O_TILES, P, J8, 2, P], f8, kind="ExternalInput")
    w16d = nc.dram_tensor("w16", [O_TILES, P, KF16, P], f16, kind="ExternalInput")
    biasd = nc.dram_tensor("bias", [P, O_TILES], f32, kind="ExternalInput")
    out = nc.dram_tensor("outT", [D_OUT, TOK_SHARD], f16, kind="ExternalOutput")

    n_groups = O_TILES // OT_GROUP

    with tile.TileContext(nc) as tc:
        with (
            tc.tile_pool(name="xpool", bufs=1) as xpool,
            tc.tile_pool(name="wpool", bufs=1) as wpool,
            tc.tile_pool(name="cpool", bufs=1) as cpool,
            tc.tile_pool(name="opool", bufs=8) as opool,
            tc.tile_pool(name="ppool", bufs=8, space="PSUM") as ppool,
        ):
            x8t = xpool.tile([P, J8, 2, TOK_SHARD], f8)
            x16t = xpool.tile([P, KF16, TOK_SHARD], f16)
            w8t = wpool.tile([P, O_TILES, J8, 2, P], f8)
            w16t = wpool.tile([P, O_TILES, KF16, P], f16)
            bias_t = cpool.tile([P, O_TILES], f32)

            # Partition-major views of the W inputs so a whole 4-chunk group
            # loads in ONE descriptor issue (the Sync engine issues DMA
            # descriptors serially at ~650ns each — issue count matters).
            w8r = w8d.rearrange("o p j t i -> p o j t i")
            w16r = w16d.rearrange("o p k i -> p o k i")

            def load_w_group(g):
                sl = ds(g * OT_GROUP, OT_GROUP)
                nc.sync.dma_start(out=w8t[:, sl, :, :, :], in_=w8r[:, sl, :, :, :])
                nc.sync.dma_start(out=w16t[:, sl, :, :], in_=w16r[:, sl, :, :])

            # DMA issue order == PE consumption order (HWDGE is FIFO):
            # group-0 fp8 weights, x8 plane-by-plane, group-0 fp16 weights,
            # x16 plane-by-plane, bias, then the remaining W groups.
            nc.sync.dma_start(
                out=w8t[:, ds(0, 1), :, :, :], in_=w8r[:, ds(0, 1), :, :, :]
            )
            nc.sync.dma_start(
                out=x8t[:, 0, :, ts(0, T_CHUNK)], in_=x8d[:, 0, :, ts(0, T_CHUNK)]
            )
            nc.sync.dma_start(
                out=x8t[:, 0, :, ts(1, T_CHUNK)], in_=x8d[:, 0, :, ts(1, T_CHUNK)]
            )
            nc.sync.dma_start(
                out=w8t[:, ds(1, OT_GROUP - 1), :, :, :],
                in_=w8r[:, ds(1, OT_GROUP - 1), :, :, :],
            )
            for j in range(1, J8):
                nc.sync.dma_start(out=x8t[:, j, :, :], in_=x8d[:, j, :, :])
            nc.sync.dma_start(
                out=w16t[:, ds(0, OT_GROUP), :, :], in_=w16r[:, ds(0, OT_GROUP), :, :]
            )
            for k in range(KF16):
                nc.sync.dma_start(out=x16t[:, k, :], in_=x16d[:, k, :])
            nc.sync.dma_start(out=bias_t[:], in_=biasd[:, :])
            for g in range(1, n_groups):
                load_w_group(g)

            def mm8(ps, ot, j, tt, start):
                nc.tensor.matmul(
                    ps[:],
                    w8t[:, ot, j, :, :],
                    x8t[:, j, :, ts(tt, T_CHUNK)],
                    start=start,
                    stop=False,
                    perf_mode=DR,
                )

            def mm16(ps, ot, k, tt):
                nc.tensor.matmul(
                    ps[:],
                    w16t[:, ot, k, :],
                    x16t[:, k, ts(tt, T_CHUNK)],
                    start=False,
                    stop=(k == KF16 - 1),
                )

            def evict(ps, st, ot, tt):
                nc.scalar.activation(
                    st[:, ts(tt, T_CHUNK)],
                    ps[:],
                    IDENT,
                    bias=bias_t[:, ds(ot, 1)],
                    scale=EVICT_SCALE,
                )

            for g in range(n_groups):
                tiles = [
                    (ot, tt)
                    for ot in range(g * OT_GROUP, (g + 1) * OT_GROUP)
                    for tt in range(T_CHUNKS)
                ]
                if g == 0:
                    # k-plane-outer: all 8 tiles advance together through
                    # the operand stream, consuming each freshly arrived x
                    # plane with 8 back-to-back matmuls (DMA-paced phase).
                    pss = [
                        ppool.tile([P, T_CHUNK], f32, tag="ps", name=f"ps{i}")
                        for i in range(len(tiles))
                    ]
                    for j in range(J8):
                        for i, (ot, tt) in enumerate(tiles):
                            mm8(pss[i], ot, j, tt, start=(j == 0))
                    for k in range(KF16):
                        for i, (ot, tt) in enumerate(tiles):
                            mm16(pss[i], ot, k, tt)
                    stage = {}
                    for i, (ot, tt) in enumerate(tiles):
                        if ot not in stage:
                            stage[ot] = opool.tile([P, TOK_SHARD], f16, name="ot")
                        evict(pss[i], stage[ot], ot, tt)
                    for ot, st in stage.items():
                        nc.sync.dma_start(out=out[ts(ot, P), :], in_=st[:])
                else:
                    # tile-major: each psum tile completes as early as
                    # possible so evictions and output stores stagger into
                    # the matmul stream instead of bunching at the end.
                    for ot in range(g * OT_GROUP, (g + 1) * OT_GROUP):
                        st = opool.tile([P, TOK_SHARD], f16, name="ot")
                        last = ot == O_TILES - 1
                        for tt in range(T_CHUNKS):
                            ps = ppool.tile([P, T_CHUNK], f32, tag="ps", name="ps")
                            for j in range(J8):
                                mm8(ps, ot, j, tt, start=(j == 0))
                            for k in range(KF16):
                                mm16(ps, ot, k, tt)
                            evict(ps, st, ot, tt)
                            if last:
                                # final tile: store each half as soon as it
                                # evicts so the NEFF-end queue drain waits on
                                # a 256KB transfer, not 512KB
                                nc.sync.dma_start(
                                    out=out[ts(ot, P), ts(tt, T_CHUNK)],
                                    in_=st[:, ts(tt, T_CHUNK)],
                                )
                        if not last:
                            nc.sync.dma_start(out=out[ts(ot, P), :], in_=st[:])

    nc.compile()
    return nc


def _make_in_maps(x, W, b, lora_A, lora_B):
    # LoRA merge: W' = W + scaling * B @ A  (exact fp32 host math)
    w_merged = W + SCALING * (lora_B @ lora_A)

    KC = KF8 * P  # k cut point
    ws = w_merged.T * SW  # [D_IN, D_OUT]
    w8 = np.ascontiguousarray(
        ws[:KC].astype(E4M3).reshape(J8, 2, P, O_TILES, P).transpose(3, 2, 0, 1, 4)
    )
    w16 = np.ascontiguousarray(
        ws[KC:].astype(np.float16).reshape(KF16, P, O_TILES, P).transpose(2, 1, 0, 3)
    )

    xs = x.reshape(TOK, D_IN).T * SX  # [D_IN, TOK]
    xq8 = xs[:KC].astype(E4M3)
    xq16 = xs[KC:].astype(np.float16)

    bias = np.ascontiguousarray(b.reshape(O_TILES, P).T)  # [P, O_TILES]

    def shard8(i):
        s = xq8[:, i * TOK_SHARD : (i + 1) * TOK_SHARD]
        return np.ascontiguousarray(
            s.reshape(J8, 2, P, TOK_SHARD).transpose(2, 0, 1, 3)
        )

    def shard16(i):
        s = xq16[:, i * TOK_SHARD : (i + 1) * TOK_SHARD]
        return np.ascontiguousarray(
            s.reshape(KF16, P, TOK_SHARD).transpose(1, 0, 2)
        )

    return [
        {
            "x8": shard8(i),
            "x16": shard16(i),
            "w8": w8,
            "w16": w16,
            "bias": bias,
        }
        for i in range(N_CORES)
    ]


def kernel(x, W, b, lora_A, lora_B):
    x = np.asarray(x, dtype=np.float32)
    W = np.asarray(W, dtype=np.float32)
    b = np.asarray(b, dtype=np.float32)
    lora_A = np.asarray(lora_A, dtype=np.float32)
    lora_B = np.asarray(lora_B, dtype=np.float32)

    if "main" not in _nc_cache:
        _nc_cache["main"] = _build()
    nc = _nc_cache["main"]

    in_maps = _make_in_maps(x, W, b, lora_A, lora_B)
    res = run_bass_kernel_spmd(nc, in_maps, list(range(N_CORES)))
    out = np.concatenate(
        [res.results[i]["outT"].astype(np.float32).T for i in range(N_CORES)],
        axis=0,
    )
    return np.ascontiguousarray(out).reshape(B, S, D_OUT)


# revision 14
# speedup vs baseline: 1.0342x; 1.0209x over previous
"""LinearWithLoRA on 8 TRN2 NeuronCores — split-K fp8-DoubleRow/fp16 hybrid.

y = x @ W.T + b + 2.0 * (x @ A.T) @ B.T
  x: [4, 2048, 2048] f32, W: [2048, 2048], b: [2048], A: [16, 2048], B: [2048, 16]

Strategy:
- LoRA merge on host: W' = W + 2.0 * B @ A, so the device computes
  x @ W'.T + b. Data-parallel over tokens (8192 -> 1024/core), W'
  replicated, no collectives.
- The PE's fp8 DoubleRow mode (both operands e4m3) retires 2 k-planes per
  512-cycle instruction = 2x the bf16/fp16/fp32r rate. Pure fp8 is too
  noisy for the 2e-2 gate (2.41e-2), so split the contraction: 10 of the
  16 k-planes run as 5 DoubleRow fp8 instructions, the remaining 6 planes
  run exact in fp16. Same-scale quantization (x*32, W'*1024, powers of
  two so dequant is exact) lets both parts accumulate into one PSUM bank.
  Measured rel err 1.906e-2; PE cost = (5 + 6)/16 = 0.69 of a full-K
  one-dtype pass (~76us/core vs 109us floor for any single-dtype scheme).
- Operand swap: W chunks are stationary, x moving, so PSUM tiles are
  [out_ch(partition), tokens(free)] and the bias varies along partitions:
  eviction is ONE scalar-engine op per tile,
  out = Identity(psum * 2^-15 + bias[p]), with the vector engine left
  idle. Output is stored as f16 (halves output DMA; adds ~1e-5 rel err),
  transposed, and upcast/re-transposed on host.
- All of W' (fp8+fp16 halves, ~5.5 MiB) and the x shard stay resident in
  SBUF. Matmuls are issued k-plane-outer over groups of 8 PSUM tiles so
  the PE consumes operands in exact DMA arrival order: w8 chunks for the
  first group, then x8 plane-by-plane, then w16/x16 — the PE starts after
  ~0.75 MiB instead of waiting for the full 8.25 MiB input stream.
"""

import numpy as np
import ml_dtypes

import concourse.bass as bass
import concourse.mybir as mybir
import concourse.tile as tile
from concourse import bacc
from concourse.bass import ds, ts
from concourse.bass_utils import run_bass_kernel_spmd

B, S, D_IN, D_OUT, R = 4, 2048, 2048, 2048, 16
SCALING = 32.0 / 16.0
N_CORES = 8
TOK = B * S  # 8192
TOK_SHARD = TOK // N_CORES  # 1024
P = 128
KF8 = 10  # k-planes (128 each) done in fp8 DoubleRow (must be even)
KF16 = 16 - KF8  # k-planes done in fp16
J8 = KF8 // 2  # DoubleRow instructions per tile
T_CHUNK = 512  # psum bank: 512 f32 per partition
T_CHUNKS = TOK_SHARD // T_CHUNK  # 2
O_TILES = D_OUT // P  # 16 out-channel tiles
OT_GROUP = 4  # out-tiles per psum group (x T_CHUNKS = 8 banks)

SX = 32.0  # x scale (2^5)
SW = 1024.0  # W scale (2^10)
EVICT_SCALE = 1.0 / (SX * SW)  # 2^-15, exact in fp32

E4M3 = ml_dtypes.float8_e4m3

_nc_cache = {}


def _build():
    f32 = mybir.dt.float32
    f8 = mybir.dt.float8e4
    f16 = mybir.dt.float16
    DR = mybir.MatmulPerfMode.DoubleRow
    IDENT = mybir.ActivationFunctionType.Identity

    nc = bacc.Bacc(None, target_bir_lowering=False)
    x8d = nc.dram_tensor("x8", [P, J8, 2, TOK_SHARD], f8, kind="ExternalInput")
    x16d = nc.dram_tensor("x16", [P, KF16, TOK_SHARD], f16, kind="ExternalInput")
    w8d = nc.dram_tensor("w8", [O_TILES, P, J8, 2, P], f8, kind="ExternalInput")
    w16d = nc.dram_tensor("w16", [O_TILES, P, KF16, P], f16, kind="ExternalInput")
    biasd = nc.dram_tensor("bias", [P, O_TILES], f32, kind="ExternalInput")
    out = nc.dram_tensor("outT", [D_OUT, TOK_SHARD], f16, kind="ExternalOutput")

    n_groups = O_TILES // OT_GROUP

    with tile.TileContext(nc) as tc:
        with (
            tc.tile_pool(name="xpool", bufs=1) as xpool,
            tc.tile_pool(name="wpool", bufs=1) as wpool,
            tc.tile_pool(name="cpool", bufs=1) as cpool,
            tc.tile_pool(name="opool", bufs=8) as opool,
            tc.tile_pool(name="ppool", bufs=8, space="PSUM") as ppool,
        ):
            x8t = xpool.tile([P, J8, 2, TOK_SHARD], f8)
            x16t = xpool.tile([P, KF16, TOK_SHARD], f16)
            w8t = wpool.tile([P, O_TILES, J8, 2, P], f8)
            w16t = wpool.tile([P, O_TILES, KF16, P], f16)
            bias_t = cpool.tile([P, O_TILES], f32)

            # Partition-major views of the W inputs so a whole 4-chunk group
            # loads in ONE descriptor issue (the Sync engine issues DMA
            # descriptors serially at ~650ns each — issue count matters).
            w8r = w8d.rearrange("o p j t i -> p o j t i")
            w16r = w16d.rearrange("o p k i -> p o k i")

            def load_w_group(g):
                sl = ds(g * OT_GROUP, OT_GROUP)
                nc.sync.dma_start(out=w8t[:, sl, :, :, :], in_=w8r[:, sl, :, :, :])
                nc.sync.dma_start(out=w16t[:, sl, :, :], in_=w16r[:, sl, :, :])

            # DMA issue order == PE consumption order (HWDGE is FIFO):
            # group-0 fp8 weights, x8 plane-by-plane, group-0 fp16 weights,
            # x16 plane-by-plane, bias, then the remaining W groups.
            nc.sync.dma_start(
                out=w8t[:, ds(0, 1), :, :, :], in_=w8r[:, ds(0, 1), :, :, :]
            )
            nc.sync.dma_start(
                out=x8t[:, 0, :, ts(0, T_CHUNK)], in_=x8d[:, 0, :, ts(0, T_CHUNK)]
            )
            nc.sync.dma_start(
                out=x8t[:, 0, :, ts(1, T_CHUNK)], in_=x8d[:, 0, :, ts(1, T_CHUNK)]
            )
            nc.sync.dma_start(
                out=w8t[:, ds(1, OT_GROUP - 1), :, :, :],
                in_=w8r[:, ds(1, OT_GROUP - 1), :, :, :],
            )
            for j in range(1, J8):
                nc.sync.dma_start(out=x8t[:, j, :, :], in_=x8d[:, j, :, :])
            nc.sync.dma_start(
                out=w16t[:, ds(0, OT_GROUP), :, :], in_=w16r[:, ds(0, OT_GROUP), :, :]
            )
            for k in range(KF16):
                nc.sync.dma_start(out=x16t[:, k, :], in_=x16d[:, k, :])
            nc.sync.dma_start(out=bias_t[:], in_=biasd[:, :])
            for g in range(1, n_groups):
                load_w_group(g)

            def mm8(ps, ot, j, tt, start):
                nc.tensor.matmul(
                    ps[:],
                    w8t[:, ot, j, :, :],
                    x8t[:, j, :, ts(tt, T_CHUNK)],
                    start=start,
                    stop=False,
                    perf_mode=DR,
                )

            def mm16(ps, ot, k, tt):
                nc.tensor.matmul(
                    ps[:],
                    w16t[:, ot, k, :],
                    x16t[:, k, ts(tt, T_CHUNK)],
                    start=False,
                    stop=(k == KF16 - 1),
                )

            def evict(ps, st, ot, tt):
                nc.scalar.activation(
                    st[:, ts(tt, T_CHUNK)],
                    ps[:],
                    IDENT,
                    bias=bias_t[:, ds(ot, 1)],
                    scale=EVICT_SCALE,
                )

            # PE p-state warm-up: the clock ramps with sustained PE
            # activity (~3us). Run throwaway DoubleRow matmuls on garbage
            # tiles during the prologue DMA wait so real matmuls start at
            # full clock. Each is its own start/stop group; the first real
            # matmul re-starts its bank, wiping the garbage.
            warm_s = cpool.tile([P, 2, P], f8)
            warm_m = cpool.tile([P, 2, T_CHUNK], f8)
            warm_p = ppool.tile([P, T_CHUNK], f32, tag="ps", name="warm")
            nc.vector.memset(warm_s[:], 0)
            nc.vector.memset(warm_m[:], 0)
            for _ in range(14):
                nc.tensor.matmul(
                    warm_p[:], warm_s[:], warm_m[:],
                    start=True, stop=True, perf_mode=DR,
                    skip_group_check=True,
                )

            for g in range(n_groups):
                tiles = [
                    (ot, tt)
                    for ot in range(g * OT_GROUP, (g + 1) * OT_GROUP)
                    for tt in range(T_CHUNKS)
                ]
                if g == 0:
                    # k-plane-outer: all 8 tiles advance together through
                    # the operand stream, consuming each freshly arrived x
                    # plane with 8 back-to-back matmuls (DMA-paced phase).
                    pss = [
                        ppool.tile([P, T_CHUNK], f32, tag="ps", name=f"ps{i}")
                        for i in range(len(tiles))
                    ]
                    for j in range(J8):
                        for i, (ot, tt) in enumerate(tiles):
                            mm8(pss[i], ot, j, tt, start=(j == 0))
                    for k in range(KF16):
                        for i, (ot, tt) in enumerate(tiles):
                            mm16(pss[i], ot, k, tt)
                    stage = {}
                    for i, (ot, tt) in enumerate(tiles):
                        if ot not in stage:
                            stage[ot] = opool.tile([P, TOK_SHARD], f16, name="ot")
                        evict(pss[i], stage[ot], ot, tt)
                    for ot, st in stage.items():
                        nc.sync.dma_start(out=out[ts(ot, P), :], in_=st[:])
                else:
                    # tile-major: each psum tile completes as early as
                    # possible so evictions and output stores stagger into
                    # the matmul stream instead of bunching at the end.
                    for ot in range(g * OT_GROUP, (g + 1) * OT_GROUP):
                        st = opool.tile([P, TOK_SHARD], f16, name="ot")
                        last = ot == O_TILES - 1
                        for tt in range(T_CHUNKS):
                            ps = ppool.tile([P, T_CHUNK], f32, tag="ps", name="ps")
                            for j in range(J8):
                                mm8(ps, ot, j, tt, start=(j == 0))
                            for k in range(KF16):
                                mm16(ps, ot, k, tt)
                            evict(ps, st, ot, tt)
                            if last:
                                # final tile: store each half as soon as it
                                # evicts so the NEFF-end queue drain waits on
                                # a 256KB transfer, not 512KB
                                nc.sync.dma_start(
                                    out=out[ts(ot, P), ts(tt, T_CHUNK)],
                                    in_=st[:, ts(tt, T_CHUNK)],
                                )
                        if not last:
                            nc.sync.dma_start(out=out[ts(ot, P), :], in_=st[:])

    nc.compile()
    return nc


def _make_in_maps(x, W, b, lora_A, lora_B):
    # LoRA merge: W' = W + scaling * B @ A  (exact fp32 host math)
    w_merged = W + SCALING * (lora_B @ lora_A)

    KC = KF8 * P  # k cut point
    ws = w_merged.T * SW  # [D_IN, D_OUT]
    w8 = np.ascontiguousarray(
        ws[:KC].astype(E4M3).reshape(J8, 2, P, O_TILES, P).transpose(3, 2, 0, 1, 4)
    )
    w16 = np.ascontiguousarray(
        ws[KC:].astype(np.float16).reshape(KF16, P, O_TILES, P).transpose(2, 1, 0, 3)
    )

    xs = x.reshape(TOK, D_IN).T * SX  # [D_IN, TOK]
    xq8 = xs[:KC].astype(E4M3)
    xq16 = xs[KC:].astype(np.float16)

    bias = np.ascontiguousarray(b.reshape(O_TILES, P).T)  # [P, O_TILES]

    def shard8(i):
        s = xq8[:, i * TOK_SHARD : (i + 1) * TOK_SHARD]
        return np.ascontiguousarray(
            s.reshape(J8, 2, P, TOK_SHARD).transpose(2, 0, 1, 3)
        )

    def shard16(i):
        s = xq16[:, i * TOK_SHARD : (i + 1) * TOK_SHARD]
        return np.ascontiguousarray(
            s.reshape(KF16, P, TOK_SHARD).transpose(1, 0, 2)
        )

    return [
        {
            "x8": shard8(i),
            "x16": shard16(i),
            "w8": w8,
            "w16": w16,
            "bias": bias,
        }
        for i in range(N_CORES)
    ]


def kernel(x, W, b, lora_A, lora_B):
    x = np.asarray(x, dtype=np.float32)
    W = np.asarray(W, dtype=np.float32)
    b = np.asarray(b, dtype=np.float32)
    lora_A = np.asarray(lora_A, dtype=np.float32)
    lora_B = np.asarray(lora_B, dtype=np.float32)

    if "main" not in _nc_cache:
        _nc_cache["main"] = _build()
    nc = _nc_cache["main"]

    in_maps = _make_in_maps(x, W, b, lora_A, lora_B)
    res = run_bass_kernel_spmd(nc, in_maps, list(range(N_CORES)))
    out = np.concatenate(
        [res.results[i]["outT"].astype(np.float32).T for i in range(N_CORES)],
        axis=0,
    )
    return np.ascontiguousarray(out).reshape(B, S, D_OUT)


# revision 15
# speedup vs baseline: 1.0357x; 1.0014x over previous
"""LinearWithLoRA on 8 TRN2 NeuronCores — split-K fp8-DoubleRow/fp16 hybrid.

y = x @ W.T + b + 2.0 * (x @ A.T) @ B.T
  x: [4, 2048, 2048] f32, W: [2048, 2048], b: [2048], A: [16, 2048], B: [2048, 16]

Strategy:
- LoRA merge on host: W' = W + 2.0 * B @ A, so the device computes
  x @ W'.T + b. Data-parallel over tokens (8192 -> 1024/core), W'
  replicated, no collectives.
- The PE's fp8 DoubleRow mode (both operands e4m3) retires 2 k-planes per
  512-cycle instruction = 2x the bf16/fp16/fp32r rate. Pure fp8 is too
  noisy for the 2e-2 gate (2.41e-2), so split the contraction: 10 of the
  16 k-planes run as 5 DoubleRow fp8 instructions, the remaining 6 planes
  run exact in fp16. Same-scale quantization (x*32, W'*1024, powers of
  two so dequant is exact) lets both parts accumulate into one PSUM bank.
  Measured rel err 1.906e-2; PE cost = (5 + 6)/16 = 0.69 of a full-K
  one-dtype pass (~76us/core vs 109us floor for any single-dtype scheme).
- Operand swap: W chunks are stationary, x moving, so PSUM tiles are
  [out_ch(partition), tokens(free)] and the bias varies along partitions:
  eviction is ONE scalar-engine op per tile,
  out = Identity(psum * 2^-15 + bias[p]), with the vector engine left
  idle. Output is stored as f16 (halves output DMA; adds ~1e-5 rel err),
  transposed, and upcast/re-transposed on host.
- All of W' (fp8+fp16 halves, ~5.5 MiB) and the x shard stay resident in
  SBUF. Matmuls are issued k-plane-outer over groups of 8 PSUM tiles so
  the PE consumes operands in exact DMA arrival order: w8 chunks for the
  first group, then x8 plane-by-plane, then w16/x16 — the PE starts after
  ~0.75 MiB instead of waiting for the full 8.25 MiB input stream.
"""

import numpy as np
import ml_dtypes

import concourse.bass as bass
import concourse.mybir as mybir
import concourse.tile as tile
from concourse import bacc
from concourse.bass import ds, ts
from concourse.bass_utils import run_bass_kernel_spmd

B, S, D_IN, D_OUT, R = 4, 2048, 2048, 2048, 16
SCALING = 32.0 / 16.0
N_CORES = 8
TOK = B * S  # 8192
TOK_SHARD = TOK // N_CORES  # 1024
P = 128
KF8 = 10  # k-planes (128 each) done in fp8 DoubleRow (must be even)
KF16 = 16 - KF8  # k-planes done in fp16
J8 = KF8 // 2  # DoubleRow instructions per tile
T_CHUNK = 512  # psum bank: 512 f32 per partition
T_CHUNKS = TOK_SHARD // T_CHUNK  # 2
O_TILES = D_OUT // P  # 16 out-channel tiles
OT_GROUP = 4  # out-tiles per psum group (x T_CHUNKS = 8 banks)

SX = 32.0  # x scale (2^5)
SW = 1024.0  # W scale (2^10)
EVICT_SCALE = 1.0 / (SX * SW)  # 2^-15, exact in fp32

E4M3 = ml_dtypes.float8_e4m3

_nc_cache = {}


def _build():
    f32 = mybir.dt.float32
    f8 = mybir.dt.float8e4
    f16 = mybir.dt.float16
    DR = mybir.MatmulPerfMode.DoubleRow
    IDENT = mybir.ActivationFunctionType.Identity

    nc = bacc.Bacc(None, target_bir_lowering=False)
    x8d = nc.dram_tensor("x8", [P, J8, 2, TOK_SHARD], f8, kind="ExternalInput")
    x16d = nc.dram_tensor("x16", [P, KF16, TOK_SHARD], f16, kind="ExternalInput")
    w8d = nc.dram_tensor("w8", [O_TILES, P, J8, 2, P], f8, kind="ExternalInput")
    w16d = nc.dram_tensor("w16", [O_TILES, P, KF16, P], f16, kind="ExternalInput")
    biasd = nc.dram_tensor("bias", [P, O_TILES], f32, kind="ExternalInput")
    out = nc.dram_tensor("outT", [D_OUT, TOK_SHARD], f16, kind="ExternalOutput")

    n_groups = O_TILES // OT_GROUP

    with tile.TileContext(nc) as tc:
        with (
            tc.tile_pool(name="xpool", bufs=1) as xpool,
            tc.tile_pool(name="wpool", bufs=1) as wpool,
            tc.tile_pool(name="cpool", bufs=1) as cpool,
            tc.tile_pool(name="opool", bufs=8) as opool,
            tc.tile_pool(name="ppool", bufs=8, space="PSUM") as ppool,
        ):
            x8t = xpool.tile([P, J8, 2, TOK_SHARD], f8)
            x16t = xpool.tile([P, KF16, TOK_SHARD], f16)
            w8t = wpool.tile([P, O_TILES, J8, 2, P], f8)
            w16t = wpool.tile([P, O_TILES, KF16, P], f16)
            bias_t = cpool.tile([P, O_TILES], f32)

            # Partition-major views of the W inputs so a whole 4-chunk group
            # loads in ONE descriptor issue (the Sync engine issues DMA
            # descriptors serially at ~650ns each — issue count matters).
            w8r = w8d.rearrange("o p j t i -> p o j t i")
            w16r = w16d.rearrange("o p k i -> p o k i")

            def load_w_group(g):
                sl = ds(g * OT_GROUP, OT_GROUP)
                nc.sync.dma_start(out=w8t[:, sl, :, :, :], in_=w8r[:, sl, :, :, :])
                nc.sync.dma_start(out=w16t[:, sl, :, :], in_=w16r[:, sl, :, :])

            # DMA issue order == PE consumption order (HWDGE is FIFO):
            # group-0 fp8 weights, x8 plane-by-plane, group-0 fp16 weights,
            # x16 plane-by-plane, bias, then the remaining W groups.
            # group-0 fp8 weights j-major: the opening j0 sweep over all 8
            # tiles needs only this 128KB chunk (all 4 ot, plane j0)
            nc.sync.dma_start(
                out=w8t[:, ds(0, OT_GROUP), ds(0, 1), :, :],
                in_=w8r[:, ds(0, OT_GROUP), ds(0, 1), :, :],
            )
            nc.sync.dma_start(
                out=x8t[:, 0, :, ts(0, T_CHUNK)], in_=x8d[:, 0, :, ts(0, T_CHUNK)]
            )
            nc.sync.dma_start(
                out=x8t[:, 0, :, ts(1, T_CHUNK)], in_=x8d[:, 0, :, ts(1, T_CHUNK)]
            )
            nc.sync.dma_start(
                out=w8t[:, ds(0, OT_GROUP), ds(1, J8 - 1), :, :],
                in_=w8r[:, ds(0, OT_GROUP), ds(1, J8 - 1), :, :],
            )
            for j in range(1, J8):
                nc.sync.dma_start(out=x8t[:, j, :, :], in_=x8d[:, j, :, :])
            nc.sync.dma_start(
                out=w16t[:, ds(0, OT_GROUP), :, :], in_=w16r[:, ds(0, OT_GROUP), :, :]
            )
            for k in range(KF16):
                nc.sync.dma_start(out=x16t[:, k, :], in_=x16d[:, k, :])
            nc.sync.dma_start(out=bias_t[:], in_=biasd[:, :])
            for g in range(1, n_groups):
                load_w_group(g)

            def mm8(ps, ot, j, tt, start):
                nc.tensor.matmul(
                    ps[:],
                    w8t[:, ot, j, :, :],
                    x8t[:, j, :, ts(tt, T_CHUNK)],
                    start=start,
                    stop=False,
                    perf_mode=DR,
                )

            def mm16(ps, ot, k, tt):
                nc.tensor.matmul(
                    ps[:],
                    w16t[:, ot, k, :],
                    x16t[:, k, ts(tt, T_CHUNK)],
                    start=False,
                    stop=(k == KF16 - 1),
                )

            def evict(ps, st, ot, tt):
                nc.scalar.activation(
                    st[:, ts(tt, T_CHUNK)],
                    ps[:],
                    IDENT,
                    bias=bias_t[:, ds(ot, 1)],
                    scale=EVICT_SCALE,
                )

            # PE p-state warm-up: the clock ramps with sustained PE
            # activity (~3us). Run throwaway DoubleRow matmuls on garbage
            # tiles during the prologue DMA wait so real matmuls start at
            # full clock. Each is its own start/stop group; the first real
            # matmul re-starts its bank, wiping the garbage.
            warm_s = cpool.tile([P, 2, P], f8)
            warm_m = cpool.tile([P, 2, T_CHUNK], f8)
            warm_p = ppool.tile([P, T_CHUNK], f32, tag="ps", name="warm")
            nc.vector.memset(warm_s[:], 0)
            nc.vector.memset(warm_m[:], 0)
            for _ in range(14):
                nc.tensor.matmul(
                    warm_p[:], warm_s[:], warm_m[:],
                    start=True, stop=True, perf_mode=DR,
                    skip_group_check=True,
                )

            for g in range(n_groups):
                tiles = [
                    (ot, tt)
                    for ot in range(g * OT_GROUP, (g + 1) * OT_GROUP)
                    for tt in range(T_CHUNKS)
                ]
                if g == 0:
                    # k-plane-outer: all 8 tiles advance together through
                    # the operand stream, consuming each freshly arrived x
                    # plane with 8 back-to-back matmuls (DMA-paced phase).
                    pss = [
                        ppool.tile([P, T_CHUNK], f32, tag="ps", name=f"ps{i}")
                        for i in range(len(tiles))
                    ]
                    for j in range(J8):
                        for i, (ot, tt) in enumerate(tiles):
                            mm8(pss[i], ot, j, tt, start=(j == 0))
                    for k in range(KF16):
                        for i, (ot, tt) in enumerate(tiles):
                            mm16(pss[i], ot, k, tt)
                    stage = {}
                    for i, (ot, tt) in enumerate(tiles):
                        if ot not in stage:
                            stage[ot] = opool.tile([P, TOK_SHARD], f16, name="ot")
                        evict(pss[i], stage[ot], ot, tt)
                    for ot, st in stage.items():
                        nc.sync.dma_start(out=out[ts(ot, P), :], in_=st[:])
                else:
                    # tile-major: each psum tile completes as early as
                    # possible so evictions and output stores stagger into
                    # the matmul stream instead of bunching at the end.
                    for ot in range(g * OT_GROUP, (g + 1) * OT_GROUP):
                        st = opool.tile([P, TOK_SHARD], f16, name="ot")
                        last = ot == O_TILES - 1
                        for tt in range(T_CHUNKS):
                            ps = ppool.tile([P, T_CHUNK], f32, tag="ps", name="ps")
                            for j in range(J8):
                                mm8(ps, ot, j, tt, start=(j == 0))
                            for k in range(KF16):
                                mm16(ps, ot, k, tt)
                            evict(ps, st, ot, tt)
                            if last:
                                # final tile: store each half as soon as it
                                # evicts so the NEFF-end queue drain waits on
                                # a 256KB transfer, not 512KB
                                nc.sync.dma_start(
                                    out=out[ts(ot, P), ts(tt, T_CHUNK)],
                                    in_=st[:, ts(tt, T_CHUNK)],
                                )
                        if not last:
                            nc.sync.dma_start(out=out[ts(ot, P), :], in_=st[:])

    nc.compile()
    return nc


def _make_in_maps(x, W, b, lora_A, lora_B):
    # LoRA merge: W' = W + scaling * B @ A  (exact fp32 host math)
    w_merged = W + SCALING * (lora_B @ lora_A)

    KC = KF8 * P  # k cut point
    ws = w_merged.T * SW  # [D_IN, D_OUT]
    w8 = np.ascontiguousarray(
        ws[:KC].astype(E4M3).reshape(J8, 2, P, O_TILES, P).transpose(3, 2, 0, 1, 4)
    )
    w16 = np.ascontiguousarray(
        ws[KC:].astype(np.float16).reshape(KF16, P, O_TILES, P).transpose(2, 1, 0, 3)
    )

    xs = x.reshape(TOK, D_IN).T * SX  # [D_IN, TOK]
    xq8 = xs[:KC].astype(E4M3)
    xq16 = xs[KC:].astype(np.float16)

    bias = np.ascontiguousarray(b.reshape(O_TILES, P).T)  # [P, O_TILES]

    def shard8(i):
        s = xq8[:, i * TOK_SHARD : (i + 1) * TOK_SHARD]
        return np.ascontiguousarray(
            s.reshape(J8, 2, P, TOK_SHARD).transpose(2, 0, 1, 3)
        )

    def shard16(i):
        s = xq16[:, i * TOK_SHARD : (i + 1) * TOK_SHARD]
        return np.ascontiguousarray(
            s.reshape(KF16, P, TOK_SHARD).transpose(1, 0, 2)
        )

    return [
        {
            "x8": shard8(i),
            "x16": shard16(i),
            "w8": w8,
            "w16": w16,
            "bias": bias,
        }
        for i in range(N_CORES)
    ]


def kernel(x, W, b, lora_A, lora_B):
    x = np.asarray(x, dtype=np.float32)
    W = np.asarray(W, dtype=np.float32)
    b = np.asarray(b, dtype=np.float32)
    lora_A = np.asarray(lora_A, dtype=np.float32)
    lora_B = np.asarray(lora_B, dtype=np.float32)

    if "main" not in _nc_cache:
        _nc_cache["main"] = _build()
    nc = _nc_cache["main"]

    in_maps = _make_in_maps(x, W, b, lora_A, lora_B)
    res = run_bass_kernel_spmd(nc, in_maps, list(range(N_CORES)))
    out = np.concatenate(
        [res.results[i]["outT"].astype(np.float32).T for i in range(N_CORES)],
        axis=0,
    )
    return np.ascontiguousarray(out).reshape(B, S, D_OUT)
